# revision 15
# baseline (speedup 1.0000x reference)
"""Trainium2 Bass kernel for nn_AlignModel.

Computes out[b, j, i] = sigmoid(simp[b,j]·w_s + orig[b,i]·w_o + bias) where
orig/simp are the two halves of prop_state[b] ([B, 2S, D] -> [B,S,D] each),
w_o = W[0,:D], w_s = W[0,D:].

Sharding: data-parallel over batch B=8 across the 8 NeuronCores. Each core:
  in  x   [4096, 512] f32  (= prop_state[b])
  in  w   [1, 1024]   f32
  in  bvec[1, 1]      f32
  out out [2048, 2048] f32 (= sigmoid(s_s[:,None] + s_o[None,:] + b))

Structure (from several NTFF-profile iterations):
  - Concurrent DMAs in a queue drain round-robin, so all transfers finish
    together at ~total/bandwidth.  The orig chunks therefore use GEOMETRIC
    sizes (1,1,2,4,8 tiles): early chunks surface quickly so the DVE
    multiply pipeline starts ~7 us sooner, while the aggregate stream
    still runs at full rate.  simp rides behind as two 2 MiB transfers;
    output stores queue after them on the same Sync HWDGE queue, keeping
    the DMA pipe continuously busy from first load to last store.
  - The orig half is consumed partition-outer (i = p*16 + n): contiguous
    per-partition input descriptors, and s_o[128,16] scatters straight
    into the broadcast row [1,2048] with tiny strided DMAs (no transpose).
  - so_row -> PSUM [128,2048] replication via 4 rank-1 PE matmuls; one
    dummy matmul per chunk (fed by that chunk's data) keeps the PE
    HAM-warm so the real matmuls run at 2.4 GHz.
  - Dot products: DVE tensor_mul + ScalarE Copy-with-accum (orig) / DVE
    tensor_reduce (simp); ScalarE is reserved for phase-2 sigmoids.
  - Each output row-tile is ONE ScalarE op
      out_t = Sigmoid(s_o_bcast + bias_col_t)   (PSUM -> SBUF)
    and row-tile pairs leave as single 2 MiB DMAs.
"""

import numpy as np

import concourse.mybir as mybir
from concourse import bacc, bass_utils
from concourse.tile import TileContext

P = 128          # partitions
D = 512          # feature dim
S = 2048         # sents
NT = S // P      # 16 tiles per half
OCHUNKS = [1, 1, 2, 4, 8]   # orig tiles per chunk (geometric pacing)
SCH = 8          # simp tiles per chunk (2 MiB)
NSC = NT // SCH
NCORES = 8
F32 = mybir.dt.float32


def _kernel_body(tc, out, x, w, bvec):
    nc = tc.nc
    # orig half, partition-outer: i = p*NT + n
    xo_re = x[0:S, :].rearrange("(p n) d -> p n d", n=NT)
    # simp half, partition-inner: j = n*P + p  (bias needs column layout)
    xs_re = x[S:2 * S, :].rearrange("(n p) d -> p n d", p=P)

    with (
        tc.tile_pool(name="consts", bufs=1) as cpool,
        tc.tile_pool(name="xin", bufs=1) as xpool,
        tc.tile_pool(name="scratch", bufs=4) as spool,
        tc.tile_pool(name="outbuf", bufs=3) as opool,
        tc.tile_pool(name="psum", bufs=1, space="PSUM") as ppool,
    ):
        # --- orig input stream: geometric chunks, all in flight at once ---
        xo_tiles = []
        n0 = 0
        for c, sz in enumerate(OCHUNKS):
            xo = xpool.tile([P, sz, D], F32, tag=f"xo{c}", name=f"xo{c}")
            nc.sync.dma_start(out=xo, in_=xo_re[:, n0:n0 + sz, :])
            xo_tiles.append(xo)
            n0 += sz

        # simp tiles (loads dispatched mid-phase-1a from the Scalar queue)
        xs_tiles = [
            xpool.tile([P, SCH, D], F32, tag=f"xs{g}", name=f"xs{g}")
            for g in range(NSC)
        ]

        # w / b replicated across partitions by zero-stride DMA (SWDGE);
        # w_o first since it gates the first multiply.
        w_bc = cpool.tile([P, 2 * D], F32, tag="wbc")
        nc.gpsimd.dma_start(out=w_bc[:, 0:D],
                            in_=w[:, 0:D].broadcast_to([P, D]))
        nc.gpsimd.dma_start(out=w_bc[:, D:2 * D],
                            in_=w[:, D:2 * D].broadcast_to([P, D]))
        b_col = cpool.tile([P, 1], F32, tag="bcol")
        nc.gpsimd.dma_start(out=b_col, in_=bvec.broadcast_to([P, 1]))
        ones_row = cpool.tile([1, P], mybir.dt.bfloat16, tag="ones")
        nc.gpsimd.memset(ones_row, 1.0)

        s_o_mat = cpool.tile([P, NT], F32, tag="somat")   # s_o[p*16+n] @ [p,n]
        s_sb_mat = cpool.tile([P, NT], F32, tag="ssmat")  # s_s + b, col t
        so_row = cpool.tile([1, S], mybir.dt.bfloat16, tag="sorow")
        sob_psum = ppool.tile([P, S], F32, tag="sob")     # s_o on every row

        # --- phase 1a: orig half -> s_o -> so_row ---
        n0 = 0
        for c, sz in enumerate(OCHUNKS):
            xo = xo_tiles[c]
            for blk in range(sz):
                t = n0 + blk
                prod = spool.tile([P, D], F32, tag="prod", name=f"po{t}")
                nc.vector.tensor_mul(out=prod, in0=xo[:, blk, :],
                                     in1=w_bc[:, 0:D])
                nc.scalar.activation(
                    prod, prod, mybir.ActivationFunctionType.Copy,
                    accum_out=s_o_mat[:, t:t + 1])
            src = s_o_mat[:, n0:n0 + sz]
            dst = so_row.rearrange("o (p n) -> o p n", n=NT)[:, :, n0:n0 + sz]
            nc.gpsimd.dma_start(out=dst, in_=src)   # SWDGE casts f32->bf16
            if c == 3:
                # dispatch simp loads now: orig stream is nearly drained, and
                # the Scalar HWDGE queue is otherwise idle until the sigmoids
                for g in range(NSC):
                    nc.scalar.dma_start(
                        out=xs_tiles[g],
                        in_=xs_re[:, g * SCH:(g + 1) * SCH, :])
            n0 += sz

        # --- broadcast s_o across partitions via rank-1 matmuls ---
        for j in range(S // 512):
            nc.tensor.matmul(sob_psum[:, j * 512:(j + 1) * 512], ones_row,
                             so_row[:, j * 512:(j + 1) * 512],
                             start=True, stop=True)

        # --- phase 1b + 2: simp half -> s_s + b, then outputs ---
        o_sb = None
        for g in range(NSC):
            xs = xs_tiles[g]
            for blk in range(SCH):
                t = g * SCH + blk
                prod = spool.tile([P, D], F32, tag="prod", name=f"ps{t}")
                nc.vector.tensor_mul(out=prod, in0=xs[:, blk, :],
                                     in1=w_bc[:, D:2 * D])
                nc.vector.tensor_reduce(
                    s_sb_mat[:, t:t + 1], prod,
                    axis=mybir.AxisListType.X, op=mybir.AluOpType.add)
            nc.vector.tensor_scalar_add(
                s_sb_mat[:, g * SCH:(g + 1) * SCH],
                s_sb_mat[:, g * SCH:(g + 1) * SCH], b_col)
            for blk in range(SCH):
                t = g * SCH + blk
                q = t % 2
                if q == 0:
                    o_sb = opool.tile([P, 2, S], F32, tag="osb",
                                      name=f"opair{t // 2}")
                nc.scalar.activation(
                    o_sb[:, q, :], sob_psum,
                    mybir.ActivationFunctionType.Sigmoid,
                    bias=s_sb_mat[:, t:t + 1],
                    scale=1.0,
                )
                if q == 1:
                    r0 = (t - 1) * P
                    dst = out[r0:r0 + 2 * P, :].rearrange(
                        "(q p) i -> p q i", p=P)
                    nc.sync.dma_start(out=dst, in_=o_sb)


def build_program():
    nc = bacc.Bacc(
        "TRN2",
        debug=False,
        target_bir_lowering=False,
        num_devices=NCORES,
    )
    x = nc.dram_tensor("x", [2 * S, D], F32, kind="ExternalInput").ap()
    w = nc.dram_tensor("w", [1, 2 * D], F32, kind="ExternalInput").ap()
    bvec = nc.dram_tensor("bvec", [1, 1], F32, kind="ExternalInput").ap()
    out = nc.dram_tensor("out", [S, S], F32, kind="ExternalOutput").ap()
    with TileContext(nc) as tc:
        _kernel_body(tc, out, x, w, bvec)
    nc.compile()
    return nc


_PROGRAM = None


def _get_program():
    global _PROGRAM
    if _PROGRAM is None:
        _PROGRAM = build_program()
    return _PROGRAM


def make_in_maps(prop_state, W, b):
    prop = np.ascontiguousarray(np.asarray(prop_state, dtype=np.float32))
    w = np.ascontiguousarray(np.asarray(W, dtype=np.float32).reshape(1, 2 * D))
    bv = np.ascontiguousarray(np.asarray(b, dtype=np.float32).reshape(1, 1))
    assert prop.shape == (NCORES, 2 * S, D), prop.shape
    return [{"x": prop[i], "w": w, "bvec": bv} for i in range(NCORES)]


def kernel(A, prop_state, W, b, _trace=False):
    nc = _get_program()
    in_maps = make_in_maps(prop_state, W, b)
    res = bass_utils.run_bass_kernel_spmd(
        nc, in_maps, core_ids=list(range(NCORES)), trace=_trace)
    out = np.stack([res.results[i]["out"] for i in range(NCORES)], axis=0)
    if _trace:
        kernel.last_results = res
    return out


# revision 16
# speedup vs baseline: 1.0524x; 1.0524x over previous
"""Trainium2 Bass kernel for nn_AlignModel.

Computes out[b, j, i] = sigmoid(simp[b,j]·w_s + orig[b,i]·w_o + bias) where
orig/simp are the two halves of prop_state[b] ([B, 2S, D] -> [B,S,D] each),
w_o = W[0,:D], w_s = W[0,D:].

Sharding: data-parallel over batch B=8 across the 8 NeuronCores. Each core:
  in  x   [4096, 512] f32  (= prop_state[b])
  in  w   [1, 1024]   f32
  in  bvec[1, 1]      f32
  out out [2048, 2048] f32 (= sigmoid(s_s[:,None] + s_o[None,:] + b))

Structure (from several NTFF-profile iterations):
  - Concurrent DMAs in a queue drain round-robin, so all transfers finish
    together at ~total/bandwidth.  The orig chunks therefore use GEOMETRIC
    sizes (1,1,2,4,8 tiles): early chunks surface quickly so the DVE
    multiply pipeline starts ~7 us sooner, while the aggregate stream
    still runs at full rate.  simp rides behind as two 2 MiB transfers;
    output stores queue after them on the same Sync HWDGE queue, keeping
    the DMA pipe continuously busy from first load to last store.
  - The orig half is consumed partition-outer (i = p*16 + n): contiguous
    per-partition input descriptors, and s_o[128,16] scatters straight
    into the broadcast row [1,2048] with tiny strided DMAs (no transpose).
  - so_row -> PSUM [128,2048] replication via 4 rank-1 PE matmuls; one
    dummy matmul per chunk (fed by that chunk's data) keeps the PE
    HAM-warm so the real matmuls run at 2.4 GHz.
  - Dot products: DVE tensor_mul + ScalarE Copy-with-accum (orig) / DVE
    tensor_reduce (simp); ScalarE is reserved for phase-2 sigmoids.
  - Each output row-tile is ONE ScalarE op
      out_t = Sigmoid(s_o_bcast + bias_col_t)   (PSUM -> SBUF)
    and row-tile pairs leave as single 2 MiB DMAs.
"""

import numpy as np

import concourse.mybir as mybir
from concourse import bacc, bass_utils
from concourse.tile import TileContext

P = 128          # partitions
D = 512          # feature dim
S = 2048         # sents
NT = S // P      # 16 tiles per half
OCHUNKS = [1, 1, 2, 4, 8]   # orig tiles per chunk (geometric pacing)
SCH = 8          # simp tiles per chunk (2 MiB)
NSC = NT // SCH
NCORES = 8
F32 = mybir.dt.float32


def _kernel_body(tc, out, x, w, bvec):
    nc = tc.nc
    # orig half, partition-outer: i = p*NT + n
    xo_re = x[0:S, :].rearrange("(p n) d -> p n d", n=NT)
    # simp half, partition-inner: j = n*P + p  (bias needs column layout)
    xs_re = x[S:2 * S, :].rearrange("(n p) d -> p n d", p=P)

    with (
        tc.tile_pool(name="consts", bufs=1) as cpool,
        tc.tile_pool(name="xin", bufs=1) as xpool,
        tc.tile_pool(name="scratch", bufs=4) as spool,
        tc.tile_pool(name="outbuf", bufs=3) as opool,
        tc.tile_pool(name="psum", bufs=1, space="PSUM") as ppool,
    ):
        # --- orig input stream: geometric chunks, all in flight at once ---
        xo_tiles = []
        n0 = 0
        for c, sz in enumerate(OCHUNKS):
            xo = xpool.tile([P, sz, D], F32, tag=f"xo{c}", name=f"xo{c}")
            nc.sync.dma_start(out=xo, in_=xo_re[:, n0:n0 + sz, :])
            xo_tiles.append(xo)
            n0 += sz

        # simp tiles; their loads go on the Sync queue behind the orig
        # chunks but are gated by a tiny DVE write into each tile (WAW dep)
        # so the transfers only start once phase 1a is nearly done -- an
        # ungated DMA would be scheduled at t=0 and starve the orig stream.
        xs_tiles = [
            xpool.tile([P, SCH, D], F32, tag=f"xs{g}", name=f"xs{g}")
            for g in range(NSC)
        ]

        # w / b replicated across partitions by zero-stride DMA (SWDGE);
        # w_o first since it gates the first multiply.
        w_bc = cpool.tile([P, 2 * D], F32, tag="wbc")
        nc.gpsimd.dma_start(out=w_bc[:, 0:D],
                            in_=w[:, 0:D].broadcast_to([P, D]))
        nc.gpsimd.dma_start(out=w_bc[:, D:2 * D],
                            in_=w[:, D:2 * D].broadcast_to([P, D]))
        b_col = cpool.tile([P, 1], F32, tag="bcol")
        nc.gpsimd.dma_start(out=b_col, in_=bvec.broadcast_to([P, 1]))
        ones_row = cpool.tile([1, P], mybir.dt.float16, tag="ones")
        nc.gpsimd.memset(ones_row, 1.0)

        s_o_mat = cpool.tile([P, NT], F32, tag="somat")   # s_o[p*16+n] @ [p,n]
        s_sb_mat = cpool.tile([P, NT], F32, tag="ssmat")  # s_s + b, col t
        so_row = cpool.tile([1, S], mybir.dt.float16, tag="sorow")
        sob_psum = ppool.tile([P, S], F32, tag="sob")     # s_o on every row

        # --- phase 1a: orig half -> s_o -> so_row ---
        n0 = 0
        for c, sz in enumerate(OCHUNKS):
            xo = xo_tiles[c]
            for blk in range(sz):
                t = n0 + blk
                prod = spool.tile([P, D], F32, tag="prod", name=f"po{t}")
                nc.vector.tensor_mul(out=prod, in0=xo[:, blk, :],
                                     in1=w_bc[:, 0:D])
                nc.scalar.activation(
                    prod, prod, mybir.ActivationFunctionType.Copy,
                    accum_out=s_o_mat[:, t:t + 1])
                if t == 10 or t == 12:
                    g = 0 if t == 10 else 1
                    nc.vector.tensor_copy(out=xs_tiles[g][0:1, 0, 0:1],
                                          in_=w_bc[0:1, 0:1])
            src = s_o_mat[:, n0:n0 + sz]
            dst = so_row.rearrange("o (p n) -> o p n", n=NT)[:, :, n0:n0 + sz]
            nc.gpsimd.dma_start(out=dst, in_=src)   # SWDGE casts f32->fp16
            n0 += sz

        # simp loads: queued on Sync behind the orig chunks, released by the
        # gate writes above
        for g in range(NSC):
            nc.sync.dma_start(out=xs_tiles[g],
                              in_=xs_re[:, g * SCH:(g + 1) * SCH, :])

        # --- broadcast s_o across partitions via rank-1 matmuls ---
        for j in range(S // 512):
            nc.tensor.matmul(sob_psum[:, j * 512:(j + 1) * 512], ones_row,
                             so_row[:, j * 512:(j + 1) * 512],
                             start=True, stop=True)

        # --- phase 1b + 2: simp half -> s_s + b, then outputs ---
        o_sb = None
        for g in range(NSC):
            xs = xs_tiles[g]
            for blk in range(SCH):
                t = g * SCH + blk
                prod = spool.tile([P, D], F32, tag="prod", name=f"ps{t}")
                nc.vector.tensor_mul(out=prod, in0=xs[:, blk, :],
                                     in1=w_bc[:, D:2 * D])
                nc.vector.tensor_reduce(
                    s_sb_mat[:, t:t + 1], prod,
                    axis=mybir.AxisListType.X, op=mybir.AluOpType.add)
            nc.vector.tensor_scalar_add(
                s_sb_mat[:, g * SCH:(g + 1) * SCH],
                s_sb_mat[:, g * SCH:(g + 1) * SCH], b_col)
            for blk in range(SCH):
                t = g * SCH + blk
                q = t % 2
                if q == 0:
                    o_sb = opool.tile([P, 2, S], F32, tag="osb",
                                      name=f"opair{t // 2}")
                nc.scalar.activation(
                    o_sb[:, q, :], sob_psum,
                    mybir.ActivationFunctionType.Sigmoid,
                    bias=s_sb_mat[:, t:t + 1],
                    scale=1.0,
                )
                if q == 1:
                    r0 = (t - 1) * P
                    dst = out[r0:r0 + 2 * P, :].rearrange(
                        "(q p) i -> p q i", p=P)
                    nc.sync.dma_start(out=dst, in_=o_sb)


def build_program():
    nc = bacc.Bacc(
        "TRN2",
        debug=False,
        target_bir_lowering=False,
        num_devices=NCORES,
    )
    x = nc.dram_tensor("x", [2 * S, D], F32, kind="ExternalInput").ap()
    w = nc.dram_tensor("w", [1, 2 * D], F32, kind="ExternalInput").ap()
    bvec = nc.dram_tensor("bvec", [1, 1], F32, kind="ExternalInput").ap()
    out = nc.dram_tensor("out", [S, S], F32, kind="ExternalOutput").ap()
    with TileContext(nc) as tc:
        _kernel_body(tc, out, x, w, bvec)
    nc.compile()
    return nc


_PROGRAM = None


def _get_program():
    global _PROGRAM
    if _PROGRAM is None:
        _PROGRAM = build_program()
    return _PROGRAM


def make_in_maps(prop_state, W, b):
    prop = np.ascontiguousarray(np.asarray(prop_state, dtype=np.float32))
    w = np.ascontiguousarray(np.asarray(W, dtype=np.float32).reshape(1, 2 * D))
    bv = np.ascontiguousarray(np.asarray(b, dtype=np.float32).reshape(1, 1))
    assert prop.shape == (NCORES, 2 * S, D), prop.shape
    return [{"x": prop[i], "w": w, "bvec": bv} for i in range(NCORES)]


def kernel(A, prop_state, W, b, _trace=False):
    nc = _get_program()
    in_maps = make_in_maps(prop_state, W, b)
    res = bass_utils.run_bass_kernel_spmd(
        nc, in_maps, core_ids=list(range(NCORES)), trace=_trace)
    out = np.stack([res.results[i]["out"] for i in range(NCORES)], axis=0)
    if _trace:
        kernel.last_results = res
    return out


# revision 18
# speedup vs baseline: 1.0762x; 1.0226x over previous
"""Trainium2 Bass kernel for nn_AlignModel.

Computes out[b, j, i] = sigmoid(simp[b,j]·w_s + orig[b,i]·w_o + bias) where
orig/simp are the two halves of prop_state[b] ([B, 2S, D] -> [B,S,D] each),
w_o = W[0,:D], w_s = W[0,D:].

Sharding: data-parallel over batch B=8 across the 8 NeuronCores. Each core:
  in  x   [4096, 512] f32  (= prop_state[b])
  in  w   [1, 1024]   f32
  in  bvec[1, 1]      f32
  out out [2048, 2048] f32 (= sigmoid(s_s[:,None] + s_o[None,:] + b))

Structure (from several NTFF-profile iterations):
  - Concurrent DMAs in a queue drain round-robin, so all transfers finish
    together at ~total/bandwidth.  The orig chunks therefore use GEOMETRIC
    sizes (1,1,2,4,8 tiles): early chunks surface quickly so the DVE
    multiply pipeline starts ~7 us sooner, while the aggregate stream
    still runs at full rate.  simp rides behind as two 2 MiB transfers;
    output stores queue after them on the same Sync HWDGE queue, keeping
    the DMA pipe continuously busy from first load to last store.
  - The orig half is consumed partition-outer (i = p*16 + n): contiguous
    per-partition input descriptors, and s_o[128,16] scatters straight
    into the broadcast row [1,2048] with tiny strided DMAs (no transpose).
  - so_row -> PSUM [128,2048] replication via 4 rank-1 PE matmuls; one
    dummy matmul per chunk (fed by that chunk's data) keeps the PE
    HAM-warm so the real matmuls run at 2.4 GHz.
  - Dot products: DVE tensor_mul + ScalarE Copy-with-accum (orig) / DVE
    tensor_reduce (simp); ScalarE is reserved for phase-2 sigmoids.
  - Each output row-tile is ONE ScalarE op
      out_t = Sigmoid(s_o_bcast + bias_col_t)   (PSUM -> SBUF)
    and row-tile pairs leave as single 2 MiB DMAs.
"""

import numpy as np

import concourse.mybir as mybir
from concourse import bacc, bass_utils
from concourse.tile import TileContext

P = 128          # partitions
D = 512          # feature dim
S = 2048         # sents
NT = S // P      # 16 tiles per half
OCHUNKS = [1, 1, 2, 4, 8]   # orig tiles per chunk (geometric pacing)
SCH = 4          # simp tiles per chunk (1 MiB)
NSC = NT // SCH
NCORES = 8
F32 = mybir.dt.float32


def _kernel_body(tc, out, x, w, bvec):
    nc = tc.nc
    # orig half, partition-outer: i = p*NT + n
    xo_re = x[0:S, :].rearrange("(p n) d -> p n d", n=NT)
    # simp half, partition-inner: j = n*P + p  (bias needs column layout)
    xs_re = x[S:2 * S, :].rearrange("(n p) d -> p n d", p=P)

    with (
        tc.tile_pool(name="consts", bufs=1) as cpool,
        tc.tile_pool(name="xin", bufs=1) as xpool,
        tc.tile_pool(name="scratch", bufs=4) as spool,
        tc.tile_pool(name="outbuf", bufs=4) as opool,
        tc.tile_pool(name="psum", bufs=1, space="PSUM") as ppool,
    ):
        # --- orig input stream: geometric chunks, all in flight at once ---
        xo_tiles = []
        n0 = 0
        for c, sz in enumerate(OCHUNKS):
            xo = xpool.tile([P, sz, D], F32, tag=f"xo{c}", name=f"xo{c}")
            nc.sync.dma_start(out=xo, in_=xo_re[:, n0:n0 + sz, :])
            xo_tiles.append(xo)
            n0 += sz

        # simp tiles; their loads go on the Sync queue behind the orig
        # chunks but are gated by a tiny DVE write into each tile (WAW dep)
        # so the transfers only start once phase 1a is nearly done -- an
        # ungated DMA would be scheduled at t=0 and starve the orig stream.
        xs_tiles = [
            xpool.tile([P, SCH, D], F32, tag=f"xs{g}", name=f"xs{g}")
            for g in range(NSC)
        ]

        # w / b replicated across partitions by zero-stride DMA (SWDGE);
        # w_o first since it gates the first multiply.
        w_bc = cpool.tile([P, 2 * D], F32, tag="wbc")
        nc.gpsimd.dma_start(out=w_bc[:, 0:D],
                            in_=w[:, 0:D].broadcast_to([P, D]))
        nc.gpsimd.dma_start(out=w_bc[:, D:2 * D],
                            in_=w[:, D:2 * D].broadcast_to([P, D]))
        ones_row = cpool.tile([1, P], mybir.dt.float16, tag="ones")
        nc.gpsimd.memset(ones_row, 1.0)

        s_o_mat = cpool.tile([P, NT], F32, tag="somat")   # s_o[p*16+n] @ [p,n]
        s_sb_mat = cpool.tile([P, NT], F32, tag="ssmat")  # s_s + b, col t
        so_row = cpool.tile([1, S], mybir.dt.float16, tag="sorow")
        b_sb = cpool.tile([1, 1], F32, tag="bsb")
        nc.sync.dma_start(out=b_sb, in_=bvec)
        nc.gpsimd.memset(so_row, 0.0)
        nc.vector.tensor_scalar_add(so_row, so_row, b_sb)
        sob_psum = ppool.tile([P, S], F32, tag="sob")     # s_o on every row

        # --- phase 1a: orig half -> s_o -> so_row ---
        n0 = 0
        for c, sz in enumerate(OCHUNKS):
            xo = xo_tiles[c]
            for blk in range(sz):
                t = n0 + blk
                prod = spool.tile([P, D], F32, tag="prod", name=f"po{t}")
                nc.vector.tensor_mul(out=prod, in0=xo[:, blk, :],
                                     in1=w_bc[:, 0:D])
                nc.scalar.activation(
                    prod, prod, mybir.ActivationFunctionType.Copy,
                    accum_out=s_o_mat[:, t:t + 1])
                if t in (10, 12):
                    for g in ((0, 1) if t == 10 else (2, 3)):
                        nc.vector.tensor_copy(
                            out=xs_tiles[g][0:1, 0, 0:1],
                            in_=prod[0:1, 0:1])
            src = s_o_mat[:, n0:n0 + sz]
            dst = so_row.rearrange("o (p n) -> o p n", n=NT)[:, :, n0:n0 + sz]
            # SWDGE: casts f32->fp16 and accumulates onto the +b prefill
            nc.gpsimd.dma_start(out=dst, in_=src,
                                accum_op=mybir.AluOpType.add)
            n0 += sz

        # simp loads: queued on Sync behind the orig chunks, released by the
        # gate writes above
        for g in range(NSC):
            nc.sync.dma_start(out=xs_tiles[g],
                              in_=xs_re[:, g * SCH:(g + 1) * SCH, :])

        # --- broadcast s_o across partitions via rank-1 matmuls ---
        for j in range(S // 512):
            nc.tensor.matmul(sob_psum[:, j * 512:(j + 1) * 512], ones_row,
                             so_row[:, j * 512:(j + 1) * 512],
                             start=True, stop=True)

        # --- phase 1b + 2: simp half -> s_s + b, then outputs ---
        o_sb = None
        for g in range(NSC):
            xs = xs_tiles[g]
            for blk in range(SCH):
                t = g * SCH + blk
                prod = spool.tile([P, D], F32, tag="prod", name=f"ps{t}")
                nc.vector.tensor_mul(out=prod, in0=xs[:, blk, :],
                                     in1=w_bc[:, D:2 * D])
                nc.vector.tensor_reduce(
                    s_sb_mat[:, t:t + 1], prod,
                    axis=mybir.AxisListType.X, op=mybir.AluOpType.add)
            for blk in range(SCH):
                t = g * SCH + blk
                q = t % 2
                if q == 0:
                    o_sb = opool.tile([P, 2, S], F32, tag="osb",
                                      name=f"opair{t // 2}")
                nc.scalar.activation(
                    o_sb[:, q, :], sob_psum,
                    mybir.ActivationFunctionType.Sigmoid,
                    bias=s_sb_mat[:, t:t + 1],
                    scale=1.0,
                )
                if q == 1:
                    r0 = (t - 1) * P
                    dst = out[r0:r0 + 2 * P, :].rearrange(
                        "(q p) i -> p q i", p=P)
                    nc.sync.dma_start(out=dst, in_=o_sb)


def build_program():
    nc = bacc.Bacc(
        "TRN2",
        debug=False,
        target_bir_lowering=False,
        num_devices=NCORES,
    )
    x = nc.dram_tensor("x", [2 * S, D], F32, kind="ExternalInput").ap()
    w = nc.dram_tensor("w", [1, 2 * D], F32, kind="ExternalInput").ap()
    bvec = nc.dram_tensor("bvec", [1, 1], F32, kind="ExternalInput").ap()
    out = nc.dram_tensor("out", [S, S], F32, kind="ExternalOutput").ap()
    with TileContext(nc) as tc:
        _kernel_body(tc, out, x, w, bvec)
    nc.compile()
    return nc


_PROGRAM = None


def _get_program():
    global _PROGRAM
    if _PROGRAM is None:
        _PROGRAM = build_program()
    return _PROGRAM


def make_in_maps(prop_state, W, b):
    prop = np.ascontiguousarray(np.asarray(prop_state, dtype=np.float32))
    w = np.ascontiguousarray(np.asarray(W, dtype=np.float32).reshape(1, 2 * D))
    bv = np.ascontiguousarray(np.asarray(b, dtype=np.float32).reshape(1, 1))
    assert prop.shape == (NCORES, 2 * S, D), prop.shape
    return [{"x": prop[i], "w": w, "bvec": bv} for i in range(NCORES)]


def kernel(A, prop_state, W, b, _trace=False):
    nc = _get_program()
    in_maps = make_in_maps(prop_state, W, b)
    res = bass_utils.run_bass_kernel_spmd(
        nc, in_maps, core_ids=list(range(NCORES)), trace=_trace)
    out = np.stack([res.results[i]["out"] for i in range(NCORES)], axis=0)
    if _trace:
        kernel.last_results = res
    return out


# revision 19
# speedup vs baseline: 1.1670x; 1.0844x over previous
"""Trainium2 Bass kernel for nn_AlignModel.

Computes out[b, j, i] = sigmoid(simp[b,j]·w_s + orig[b,i]·w_o + bias) where
orig/simp are the two halves of prop_state[b] ([B, 2S, D] -> [B,S,D] each),
w_o = W[0,:D], w_s = W[0,D:].

Sharding: data-parallel over batch B=8 across the 8 NeuronCores. Each core:
  in  x   [4096, 512] f32  (= prop_state[b])
  in  w   [1, 1024]   f32
  in  bvec[1, 1]      f32
  out out [2048, 2048] f32 (= sigmoid(s_s[:,None] + s_o[None,:] + b))

Structure (from several NTFF-profile iterations):
  - Concurrent DMAs in a queue drain round-robin, so all transfers finish
    together at ~total/bandwidth.  The orig chunks therefore use GEOMETRIC
    sizes (1,1,2,4,8 tiles): early chunks surface quickly so the DVE
    multiply pipeline starts ~7 us sooner, while the aggregate stream
    still runs at full rate.  simp rides behind as two 2 MiB transfers;
    output stores queue after them on the same Sync HWDGE queue, keeping
    the DMA pipe continuously busy from first load to last store.
  - The orig half is consumed partition-outer (i = p*16 + n): contiguous
    per-partition input descriptors, and s_o[128,16] scatters straight
    into the broadcast row [1,2048] with tiny strided DMAs (no transpose).
  - so_row -> PSUM [128,2048] replication via 4 rank-1 PE matmuls; one
    dummy matmul per chunk (fed by that chunk's data) keeps the PE
    HAM-warm so the real matmuls run at 2.4 GHz.
  - Dot products: DVE tensor_mul + ScalarE Copy-with-accum (orig) / DVE
    tensor_reduce (simp); ScalarE is reserved for phase-2 sigmoids.
  - Each output row-tile is ONE ScalarE op
      out_t = Sigmoid(s_o_bcast + bias_col_t)   (PSUM -> SBUF)
    and row-tile pairs leave as single 2 MiB DMAs.
"""

import numpy as np

import concourse.mybir as mybir
from concourse import bacc, bass_utils
from concourse.tile import TileContext

P = 128          # partitions
D = 512          # feature dim
S = 2048         # sents
NT = S // P      # 16 tiles per half
OCHUNKS = [1, 1, 2, 4, 4, 4]   # orig tiles per chunk (geometric pacing)
SCH = 4          # simp tiles per chunk (1 MiB)
NSC = NT // SCH
NCORES = 8
F32 = mybir.dt.float32


def _kernel_body(tc, out, x, w, bvec):
    nc = tc.nc
    # orig half, partition-outer: i = p*NT + n
    xo_re = x[0:S, :].rearrange("(p n) d -> p n d", n=NT)
    # simp half, partition-inner: j = n*P + p  (bias needs column layout)
    xs_re = x[S:2 * S, :].rearrange("(n p) d -> p n d", p=P)

    with (
        tc.tile_pool(name="consts", bufs=1) as cpool,
        tc.tile_pool(name="xin", bufs=1) as xpool,
        tc.tile_pool(name="scratch", bufs=4) as spool,
        tc.tile_pool(name="outbuf", bufs=4) as opool,
        tc.tile_pool(name="psum", bufs=1, space="PSUM") as ppool,
    ):
        # --- orig input stream: geometric chunks, all in flight at once ---
        xo_tiles = []
        n0 = 0
        for c, sz in enumerate(OCHUNKS):
            xo = xpool.tile([P, sz, D], F32, tag=f"xo{c}", name=f"xo{c}")
            nc.sync.dma_start(out=xo, in_=xo_re[:, n0:n0 + sz, :])
            xo_tiles.append(xo)
            n0 += sz

        # simp tiles; their loads go on the Sync queue behind the orig
        # chunks but are gated by a tiny DVE write into each tile (WAW dep)
        # so the transfers only start once phase 1a is nearly done -- an
        # ungated DMA would be scheduled at t=0 and starve the orig stream.
        xs_tiles = [
            xpool.tile([P, SCH, D], F32, tag=f"xs{g}", name=f"xs{g}")
            for g in range(NSC)
        ]

        # w / b replicated across partitions by zero-stride DMA (SWDGE);
        # w_o first since it gates the first multiply.
        w_bc = cpool.tile([P, 2 * D], F32, tag="wbc")
        nc.gpsimd.dma_start(out=w_bc[:, 0:D],
                            in_=w[:, 0:D].broadcast_to([P, D]))
        nc.gpsimd.dma_start(out=w_bc[:, D:2 * D],
                            in_=w[:, D:2 * D].broadcast_to([P, D]))
        ones_row = cpool.tile([1, P], mybir.dt.float16, tag="ones")
        nc.gpsimd.memset(ones_row, 1.0)

        s_o_mat = cpool.tile([P, NT], F32, tag="somat")   # s_o[p*16+n] @ [p,n]
        s_sb_mat = cpool.tile([P, NT], F32, tag="ssmat")  # s_s + b, col t
        so_row = cpool.tile([1, S], mybir.dt.float16, tag="sorow")
        b_sb = cpool.tile([1, 1], F32, tag="bsb")
        nc.sync.dma_start(out=b_sb, in_=bvec)
        b_row = cpool.tile([1, 512], mybir.dt.float16, tag="brow")
        nc.gpsimd.memset(b_row, 0.0)
        nc.vector.tensor_scalar_add(b_row, b_row, b_sb)
        sob_psum = ppool.tile([P, S], F32, tag="sob")     # s_o on every row

        # --- phase 1a: orig half -> s_o -> so_row ---
        n0 = 0
        for c, sz in enumerate(OCHUNKS):
            xo = xo_tiles[c]
            for blk in range(sz):
                t = n0 + blk
                prod = spool.tile([P, D], F32, tag="prod", name=f"po{t}")
                nc.vector.tensor_mul(out=prod, in0=xo[:, blk, :],
                                     in1=w_bc[:, 0:D])
                nc.scalar.activation(
                    prod, prod, mybir.ActivationFunctionType.Copy,
                    accum_out=s_o_mat[:, t:t + 1])
                if t in (8, 10, 12):
                    gs = {8: (0,), 10: (1,), 12: (2, 3)}[t]
                    for g in gs:
                        nc.vector.tensor_copy(
                            out=xs_tiles[g][0:1, 0, 0:1],
                            in_=prod[0:1, 0:1])
            src = s_o_mat[:, n0:n0 + sz]
            dst = so_row.rearrange("o (p n) -> o p n", n=NT)[:, :, n0:n0 + sz]
            nc.gpsimd.dma_start(out=dst, in_=src)   # SWDGE casts f32->fp16
            n0 += sz

        # simp loads: queued on Sync behind the orig chunks, released by the
        # gate writes above
        for g in range(NSC):
            nc.sync.dma_start(out=xs_tiles[g],
                              in_=xs_re[:, g * SCH:(g + 1) * SCH, :])

        # --- broadcast b + s_o across partitions via rank-1 matmuls: the b
        # seed runs early (start=True), s_o accumulates on top ---
        for j in range(S // 512):
            nc.tensor.matmul(sob_psum[:, j * 512:(j + 1) * 512], ones_row,
                             b_row, start=True, stop=False)
        for j in range(S // 512):
            nc.tensor.matmul(sob_psum[:, j * 512:(j + 1) * 512], ones_row,
                             so_row[:, j * 512:(j + 1) * 512],
                             start=False, stop=True)

        # --- phase 1b + 2: simp half -> s_s + b, then outputs ---
        o_sb = None
        for g in range(NSC):
            xs = xs_tiles[g]
            for blk in range(SCH):
                t = g * SCH + blk
                prod = spool.tile([P, D], F32, tag="prod", name=f"ps{t}")
                nc.vector.tensor_mul(out=prod, in0=xs[:, blk, :],
                                     in1=w_bc[:, D:2 * D])
                nc.vector.tensor_reduce(
                    s_sb_mat[:, t:t + 1], prod,
                    axis=mybir.AxisListType.X, op=mybir.AluOpType.add)
            for blk in range(SCH):
                t = g * SCH + blk
                q = t % 2
                if q == 0:
                    o_sb = opool.tile([P, 2, S], F32, tag="osb",
                                      name=f"opair{t // 2}")
                nc.scalar.activation(
                    o_sb[:, q, :], sob_psum,
                    mybir.ActivationFunctionType.Sigmoid,
                    bias=s_sb_mat[:, t:t + 1],
                    scale=1.0,
                )
                if q == 1:
                    r0 = (t - 1) * P
                    dst = out[r0:r0 + 2 * P, :].rearrange(
                        "(q p) i -> p q i", p=P)
                    nc.sync.dma_start(out=dst, in_=o_sb)


def build_program():
    nc = bacc.Bacc(
        "TRN2",
        debug=False,
        target_bir_lowering=False,
        num_devices=NCORES,
    )
    x = nc.dram_tensor("x", [2 * S, D], F32, kind="ExternalInput").ap()
    w = nc.dram_tensor("w", [1, 2 * D], F32, kind="ExternalInput").ap()
    bvec = nc.dram_tensor("bvec", [1, 1], F32, kind="ExternalInput").ap()
    out = nc.dram_tensor("out", [S, S], F32, kind="ExternalOutput").ap()
    with TileContext(nc) as tc:
        _kernel_body(tc, out, x, w, bvec)
    nc.compile()
    return nc


_PROGRAM = None


def _get_program():
    global _PROGRAM
    if _PROGRAM is None:
        _PROGRAM = build_program()
    return _PROGRAM


def make_in_maps(prop_state, W, b):
    prop = np.ascontiguousarray(np.asarray(prop_state, dtype=np.float32))
    w = np.ascontiguousarray(np.asarray(W, dtype=np.float32).reshape(1, 2 * D))
    bv = np.ascontiguousarray(np.asarray(b, dtype=np.float32).reshape(1, 1))
    assert prop.shape == (NCORES, 2 * S, D), prop.shape
    return [{"x": prop[i], "w": w, "bvec": bv} for i in range(NCORES)]


def kernel(A, prop_state, W, b, _trace=False):
    nc = _get_program()
    in_maps = make_in_maps(prop_state, W, b)
    res = bass_utils.run_bass_kernel_spmd(
        nc, in_maps, core_ids=list(range(NCORES)), trace=_trace)
    out = np.stack([res.results[i]["out"] for i in range(NCORES)], axis=0)
    if _trace:
        kernel.last_results = res
    return out
